# revision 55
# baseline (speedup 1.0000x reference)
"""Trainium2 Bass kernel for nn_AFRM_48636209660262.

Conv-BiLSTM autoencoder: 4x strided conv encoder -> channel-split BiLSTM ->
1x1 conv ffwd -> 4x conv_transpose decoder -> gamma*h + x.

Strategy: pure data parallelism over 8 NeuronCores (4 samples each, no
collectives). Activations are channel-major [C_chunk(128), B, H, W] with
zero-padded borders; convs are per-tap matmuls accumulated in PSUM
(weight-stationary loop order so each LDWEIGHTS amortizes over 2-4 psum
tiles). All 8 conv layers run in fp8e4m3 (the four big ones - enc L1/L2,
dec L2/L3 - with perf_mode=DoubleRow, K=256 per matmul; the small middle
layers as plain fp8 with per-ko weight slices since their merged-b rhs APs
exceed the TENSOR3D pattern limit); per-layer power-of-2 weight scales are
undone via the evacuation's scale AP, BN folded into weights on the host,
bias + ReLU applied by ScalarE on evacuation. conv_transpose = 4 parity
classes x 4 taps. The enc4 -> LSTM feature shuffle happens on-chip: one
shift-by-64 permutation matmul per kc plus 8 partition-aligned DVE copies
build the DoubleRow-ready seqT stationary directly (no DRAM bounce); W rows
are host-permuted to match. z_x is precomputed for all timesteps into
per-(dir,q) tiles (evacuation alternates ScalarE/VectorE so t=0 gates start
early); the recurrence folds each round's z_x slice into the h@U psum group
with a scU-scaled 4x4 identity matmul, so gates read PSUM directly with the
shared 1/scU activation scale. h^T chunks come from small PE transposes
into per-chunk H tiles so the next round's matmuls start immediately.
Residual x and the output travel as bf16; PSUM accumulation is f32
throughout. Host-side prep only reshapes/quantizes weights and the input.
"""
import numpy as np
import ml_dtypes

import concourse.mybir as mybir
import concourse.tile as tile
from concourse import bacc
from concourse.bass_utils import run_bass_kernel_spmd
from concourse.masks import make_identity

AF = mybir.ActivationFunctionType
DR = mybir.MatmulPerfMode.DoubleRow
BF16 = mybir.dt.bfloat16
F32 = mybir.dt.float32
F8 = mybir.dt.float8e4
NPF8 = ml_dtypes.float8_e4m3fn
NPBF = ml_dtypes.bfloat16

N_CORES = 8
B = 4           # batch per core
C = 256
BN_EPS = 1e-3

_CACHE: dict = {}


def _stepped(start, count, step):
    return slice(start, start + step * (count - 1) + 1, step)


def _build(gamma_nonneg=True, use_bias=False, dbg=None):
    nc = bacc.Bacc("TRN2", target_bir_lowering=False, debug=False,
                   num_devices=N_CORES)

    xin = nc.dram_tensor("xin", [128, 2, B, 66, 66], F8, kind="ExternalInput").ap()
    xres = nc.dram_tensor("xres", [2, 128, B, 64, 64], BF16, kind="ExternalInput").ap()
    # fp8 DoubleRow weights for all 8 conv layers
    # [encL1, encL2, decL2, decL3, encL3, encL4, decL0, decL1]
    w8 = nc.dram_tensor("w8", [8, 128, 16, 2, 2, 128], F8, kind="ExternalInput").ap()
    bconv = nc.dram_tensor("bconv", [128, 34], F32, kind="ExternalInput").ap()
    # LSTM mats: [Wf, Wr, Uf, Ur] x [kc, row, 4096]
    wl = nc.dram_tensor("wl", [4, 4, 128, 2, 4096], F8, kind="ExternalInput").ap()
    bl = nc.dram_tensor("bl", [2, 16, 4096], BF16, kind="ExternalInput").ap()
    wff = nc.dram_tensor("wff", [128, 4, 2, 128], F8, kind="ExternalInput").ap()
    shm = nc.dram_tensor("shm", [128, 3, 128], BF16, kind="ExternalInput").ap()
    out = nc.dram_tensor("out", [2, 128, B, 64, 64], BF16, kind="ExternalOutput").ap()

    dbg_ap = None
    dbg_shapes = {
        'l1': [2, 128, B, 34, 34], 'l2': [2, 128, B, 18, 18],
        'l3': [2, 128, B, 10, 10], 'enc4': [2, 128, B * 16],
        'seqT': [128, 4, 2, 160], 'hs': [2, 128, 4, 2, 32],
        'd0': [2, 128, B, 6, 6], 'd1': [2, 128, B, 10, 10],
        'd2': [2, 128, B, 18, 18], 'd3': [2, 128, B, 34, 34],
    }
    if dbg is not None:
        dbg_ap = nc.dram_tensor("dbg", dbg_shapes[dbg], BF16,
                                kind="ExternalOutput").ap()

    with tile.TileContext(nc) as tc:
        _trace(nc, tc, xin, xres, w8, bconv, wl, bl, wff, shm, out,
               gamma_nonneg, use_bias, dbg, dbg_ap)
    nc.compile()
    return nc


def _trace(nc, tc, xin, xres, w8, bconv, wl, bl, wff, shm, out,
           gamma_nonneg, use_bias, dbg, dbg_ap):
    from contextlib import ExitStack

    def memset_border(t, Hp):
        nc.vector.memset(t[:, :, 0, :], 0.0)
        nc.vector.memset(t[:, :, Hp - 1, :], 0.0)
        nc.vector.memset(t[:, :, :, 0], 0.0)
        nc.vector.memset(t[:, :, :, Hp - 1], 0.0)

    # decoder parity taps: out[2m+p] <- pairs (di, k)
    ROW_TAPS = {0: [(-1, 0), (0, 2)], 1: [(0, 1), (1, 3)]}

    with ExitStack() as top:
        persist = top.enter_context(tc.tile_pool(name="persist", bufs=1))

        bias_sb = persist.tile([128, 34], F32)
        nc.sync.dma_start(bias_sb[:], bconv[:])
        warm = persist.tile([1, 2], BF16, name="warm")
        nc.scalar.activation(warm[:, 0:1], bias_sb[0:1, 0:1], AF.Sigmoid)
        nc.scalar.activation(warm[:, 1:2], bias_sb[0:1, 0:1], AF.Tanh)
        ident8 = persist.tile([128, 128], BF16)
        make_identity(nc, ident8[:])
        # shift-by-64 permutation: SH[p, (p+64)%128] = 1
        shid = persist.tile([128, 128], BF16, name="shid")
        nc.vector.tensor_copy(shid[:, 64:128], ident8[:, 0:64])
        nc.vector.tensor_copy(shid[:, 0:64], ident8[:, 64:128])
        # h^T history per dir: [c, kp, ko, u*4+b] where col band u holds
        # h_{u-1} (band 0 = initial zeros); recurrence step t reads band t,
        # ffwd reads band t+1
        # Ht[1] has a 32-col zero prefix per (kp,ko): its z_u stationary is
        # M=48 (cols 0:32 zero) so the matmul lands d1 at psum rows 32:48
        # while keeping tile position 0 (DR + col-tile offset is invalid ISA)
        Ht = [persist.tile([128, 4, 2, 160], F8, name="hT0"),
              persist.tile([128, 4, 2, 160], F8, name="hT1")]
        for d in range(2):
            nc.vector.memset(Ht[d][:], 0.0)
        # shift matrices for the z_x fold: SH[t][4t+r, r] = 1 and
        # SH[t][32+4t+r, 32+r] = 1, so one bf16 matmul adds the step-t z_x
        # rows of both dirs into the gate psum bands at rows 0:4 / 32:36
        # (host-built: engine copies can't write partition base 4t)
        SHt = persist.tile([128, 3, 128], BF16, name="shm")
        nc.sync.dma_start(SHt[:], shm[:])
        enc4 = [persist.tile([128, 16, B], F8, name=f"enc4_{kc}")
                for kc in range(2)]  # [c, (hh,ww), b]
        d0 = persist.tile([128, 2, B, 6, 6], F8, name="d0m")
        wffsb = persist.tile([128, 4, 2, 128], F8)
        for mc in range(2):
            memset_border(d0[:, mc], 6)

        # decoder weights, all fp8 DR: [decL0, decL1] here; L2/L3 live in the
        # decoder pool (frees 16KB/partition during the LSTM phase)
        w8d = [persist.tile([128, 16, 2, 2, 128], F8, name=f"w8d{l}")
               for l in range(2)]

        # lwa pool spans encoder+lstm: W matrices prefetch during L2-L4
        # via Scalar-queue-gated triggers (full DMA bandwidth stays on the
        # critical x/w8 tiles during L1)
        with tc.tile_pool(name="lwa", bufs=1) as lwp:
            WLf = [lwp.tile([128, 2, 4096], F8, tag="lwa", bufs=4,
                            name=f"wf_{kp}") for kp in range(4)]
            WLr = [lwp.tile([128, 2, 4096], F8, tag="lwb", bufs=4,
                            name=f"wr_{kp}") for kp in range(4)]

            # ================= encoder =================
            with tc.tile_pool(name="encp", bufs=1) as ep, \
                 tc.tile_pool(name="encps", bufs=1, space="PSUM") as pp:
                # fp8 DoubleRow weights [L1, L2, L3, L4] <- w8[0,1,4,5];
                # L1's entry is host-packed mc-major so the first matmul
                # group only waits on the mc=0 half of the transfer
                w8t = [ep.tile([128, 16, 2, 2, 128], F8, tag="cw8", bufs=4,
                               name=f"w8e{l}") for l in range(1, 4)]
                w8t0 = ep.tile([128, 2, 16, 2, 128], F8, tag="cw8", bufs=4,
                               name="w8e0")
                w80v = w8[0].rearrange("p a c d m -> p (a c d m)").rearrange(
                    "p (mc t ko m) -> p mc t ko m", mc=2, t=16, ko=2)
                w8t = [None] + w8t
                # x tiles split per (b, row-half), merged ko dim; first-group
                # tiles (b0/b1 row-half 0) lead the DMA queues
                xt = [[ep.tile([128, 2, 34, 66], F8, tag="xcm", bufs=8,
                               name=f"x_{b}_{hf}") for hf in range(2)]
                      for b in range(B)]
                nc.sync.dma_start(xt[0][0][:], xin[:, :, 0, 0:34, :])
                nc.sync.dma_start(xt[1][0][:], xin[:, :, 1, 0:34, :])
                nc.sync.dma_start(w8t0[:, 0], w80v[:, 0])
                nc.sync.dma_start(w8t0[:, 1], w80v[:, 1])
                for b in range(B):
                    for hf in range(2):
                        if hf == 0 and b < 2:
                            continue
                        nc.sync.dma_start(xt[b][hf][:],
                                          xin[:, :, b, 32 * hf:32 * hf + 34, :])
                nc.sync.dma_start(w8t[1][:], w8[1])

                l1 = ep.tile([128, 2, B, 34, 34], F8, tag="echain", bufs=4,
                             name="l1m")
                # l2/l3 parity-split: [c, ko, hp, wp, h', w', b] with b
                # innermost so the stride-2 conv reads collapse to 3 AP dims
                # and L3/L4 run DoubleRow with merged-b moving operands
                l2p = ep.tile([128, 2, 2, 2, 10, 10, B], F8, tag="echain",
                              bufs=4, name="l2p")
                l3p = ep.tile([128, 2, 2, 2, 6, 6, B], F8, tag="echain",
                              bufs=4, name="l3p")
                nc.vector.memset(l2p[:], 0.0)
                nc.vector.memset(l3p[:], 0.0)
                for ko in range(2):
                    memset_border(l1[:, ko], 34)

                # L1 (fp8 DR): weight-stationary, each weight streams 4
                # b-psums (2 for the first group, so the first matmuls only
                # wait on xt[0..1] + w8t[0])
                for hf, oh0 in ((0, 0), (1, 16)):
                    for mc in range(2):
                        bsets = ([(0, 1), (2, 3)] if hf == 0 and mc == 0
                                 else [(0, 1, 2, 3)])
                        for bset in bsets:
                            pss = {b: pp.tile([128, 512], F32, tag="cps",
                                              bufs=6,
                                              name=f"psl1_{hf}_{mc}_{b}")
                                   for b in bset}
                            for t in range(16):
                                kh, kw = t // 4, t % 4
                                for b in bset:
                                    rhs = xt[b][hf][:, :,
                                                   _stepped(kh, 16, 2),
                                                   _stepped(kw, 32, 2)]
                                    nc.tensor.matmul(
                                        pss[b][:], w8t0[:, mc, t, :, :], rhs,
                                        start=(t == 0), stop=(t == 15),
                                        perf_mode=DR)
                            for b in bset:
                                nc.scalar.activation(
                                    l1[:, mc, b, 1 + oh0:17 + oh0, 1:33],
                                    pss[b][:],
                                    AF.Relu, bias=bias_sb[:, mc:mc + 1],
                                    scale=bias_sb[:, 20:21])


                # deferred prefetches: issued after L1 in trace order so
                # they sit behind the critical transfers in the DMA queues
                nc.sync.dma_start(w8t[2][:], w8[4])
                nc.sync.dma_start(w8t[3][:], w8[5])
                for kp in range(4):
                    nc.sync.dma_start(WLf[kp][:], wl[0, kp])
                nc.sync.dma_start(wffsb[:], wff[:])
                for l in range(2):
                    nc.sync.dma_start(w8d[l][:], w8[[6, 7][l]])
                for kp in range(4):
                    nc.sync.dma_start(WLr[kp][:], wl[1, kp])

                # L2 (fp8 DR): per-b groups, weight-stationary over 4 b's
                for mc in range(2):
                    pss = [pp.tile([128, 256], F32, tag="cps", bufs=6,
                                   name=f"psl2_{mc}_{b}") for b in range(B)]
                    for t in range(16):
                        kh, kw = t // 4, t % 4
                        for b in range(B):
                            rhs = l1[:, :, b,
                                     _stepped(kh, 16, 2),
                                     _stepped(kw, 16, 2)]
                            nc.tensor.matmul(
                                pss[b][:], w8t[1][:, t, mc, :, :], rhs,
                                start=(t == 0), stop=(t == 15), perf_mode=DR)
                    for b in range(B):
                        psv = pss[b].rearrange(
                            "p (h a w c) -> p a c h w", h=8, a=2, w=8)
                        for eh in range(2):
                            nc.scalar.activation(
                                l2p[:, mc, eh, :, 1:9, 1:9, b], psv[:, eh],
                                AF.Relu, bias=bias_sb[:, 2 + mc:3 + mc],
                                scale=bias_sb[:, 21:22])

                # L3 (fp8 DR on parity layout): rhs [p, 2ko, h', (w'b)]
                l2f = l2p.rearrange("p k i j h w b -> p k i j h (w b)")
                for mc in range(2):
                    ps = pp.tile([128, 256], F32, tag="cps", bufs=6,
                                 name=f"psl3_{mc}")
                    for t in range(16):
                        kh, kw = t // 4, t % 4
                        rhs = l2f[:, :, 1 - kh % 2, 1 - kw % 2,
                                  (kh + 1) // 2:(kh + 1) // 2 + 8,
                                  4 * ((kw + 1) // 2):
                                  4 * ((kw + 1) // 2) + 32]
                        nc.tensor.matmul(
                            ps[:], w8t[2][:, t, mc, :, :], rhs,
                            start=(t == 0), stop=(t == 15), perf_mode=DR)
                    psv = ps.rearrange(
                        "p (h a w c b) -> p a c h w b", h=4, a=2, w=4, c=2)
                    for eh in range(2):
                        for ew in range(2):
                            nc.scalar.activation(
                                l3p[:, mc, eh, ew, 1:5, 1:5, :],
                                psv[:, eh, ew],
                                AF.Relu, bias=bias_sb[:, 4 + mc:5 + mc],
                                scale=bias_sb[:, 18:19])

                # L4 (fp8 DR on parity layout) -> enc4 [c, (hh ww), b]
                l3f = l3p.rearrange("p k i j h w b -> p k i j h (w b)")
                for mc in range(2):
                    ps = pp.tile([128, 64], F32, tag="cps", bufs=6,
                                 name=f"psl4_{mc}")
                    for t in range(16):
                        kh, kw = t // 4, t % 4
                        rhs = l3f[:, :, 1 - kh % 2, 1 - kw % 2,
                                  (kh + 1) // 2:(kh + 1) // 2 + 4,
                                  4 * ((kw + 1) // 2):
                                  4 * ((kw + 1) // 2) + 16]
                        nc.tensor.matmul(
                            ps[:], w8t[3][:, t, mc, :, :], rhs,
                            start=(t == 0), stop=(t == 15), perf_mode=DR)
                    nc.scalar.activation(
                        enc4[mc].rearrange("p hw b -> p (hw b)"),
                        ps[:], AF.Relu,
                        bias=bias_sb[:, 6 + mc:7 + mc],
                        scale=bias_sb[:, 19:20])

            if dbg == 'enc4':
                for kc in range(2):
                    nc.sync.dma_start(
                        dbg_ap[kc],
                        enc4[kc].rearrange("p hw b -> p (hw b)"))

            # ================= LSTM =================
            with tc.tile_pool(name="lstmp", bufs=1) as lp, \
                 tc.tile_pool(name="lstmps", bufs=1, space="PSUM") as lps:
                # on-chip enc4 -> seqT shuffle.  seqT row (band*64+cc) of
                # chunk (kp, ko) holds feature (hw=4kp+2band+ko, cc); cols
                # are (s,b) = (2kc+shi)*4+b.  W rows host-permuted to match.
                # Crossed half (shi != band) reads a 64-partition-swapped
                # copy made by one permutation matmul per kc.
                seqTm = lp.tile([128, 4, 2, 160], F8, name="seqTm")
                nc.vector.memset(seqTm[:], 0.0)
                e4sw = [lps.tile([128, 64], F32, tag="ptr", bufs=2,
                                 name=f"e4sw{kc}") for kc in range(2)]
                for kc in range(2):
                    nc.tensor.matmul(
                        e4sw[kc][:], shid[:],
                        enc4[kc].rearrange("p hw b -> p (hw b)"),
                        start=True, stop=True)
                for kc in range(2):
                    e4swv = e4sw[kc].rearrange("p (hw b) -> p hw b", b=B)
                    for band in range(2):
                        for am in range(2):
                            shi = band if am == 0 else 1 - band
                            s = 2 * kc + shi
                            src = (enc4[kc] if am == 0 else e4swv)
                            srcv = src[band * 64:(band + 1) * 64].rearrange(
                                "p (kp two ko) b -> p kp two ko b",
                                kp=4, two=2)[:, :, band, :, :]
                            dst = seqTm[band * 64:(band + 1) * 64, :, :,
                                        32 + s * 4:32 + s * 4 + 4]
                            # alternate engines so the 8 copies pipeline
                            # (z_x can't start until seqT is complete)
                            if (band + am) % 2 == 0:
                                nc.vector.tensor_copy(dst, srcv)
                            else:
                                nc.scalar.copy(dst, srcv)
                seqT = [seqTm[:, kp] for kp in range(4)]
                if dbg == 'seqT':
                    nc.sync.dma_start(dbg_ap[:], seqTm[:])

                # U matrices: dedicated buffers streamed during z_x / t0
                # gates. Triggered from the Scalar queue so the transfers
                # can't start before the encoder's last evacuation (running
                # them during the encoder slows its conv matmuls via SBUF
                # write contention).
                ULf = [lp.tile([128, 2, 4096], F8, tag="ula", bufs=4,
                               name=f"uf_{kp}") for kp in range(4)]
                ULr = [lp.tile([128, 2, 4096], F8, tag="ulb", bufs=4,
                               name=f"ur_{kp}") for kp in range(4)]
                UL = [ULf, ULr]
                nc.gpsimd.tensor_copy(warm[:, 1:2], enc4[0][0:1, 0, 0:1])
                for q in range(4):
                    for d in range(2):
                        for kp in range(4):
                            nc.gpsimd.dma_start(
                                UL[d][kp][:, :, q * 1024:(q + 1) * 1024],
                                wl[2 + d, kp][:, :, q * 1024:(q + 1) * 1024])

                # ---- z_x for all steps. Both dirs share one psum tile
                # (d0 rows 0:16, d1 rows 32:48 via matmul tile position), so
                # every evac/gate op covers both dirs in one instruction.
                # zxq holds RAW (sc-scaled) values; the 1/sc happens at the
                # gate activations.
                blt = None
                if use_bias:
                    blt = lp.tile([48, 4096], BF16, tag="zxbias", bufs=1,
                                  name="blt")
                    for d in range(2):
                        nc.sync.dma_start(blt[32 * d:32 * d + 16, :], bl[d])
                zxq = [lp.tile([128, 1024], BF16, tag="zxj", bufs=6,
                               name=f"zx{q}") for q in range(4)]
                for q in range(4):
                    nc.vector.memset(zxq[q][:], 0.0)
                scinv = bias_sb[0:36, 26:27]

                def zx_q(q, evac_eng):
                    ps = lps.tile([128, 1024], F32, tag="pz", bufs=3,
                                  name=f"pzx{q}")
                    for d in (0, 1):
                        WT = (WLf, WLr)[d]
                        for kp in range(4):
                            stat = (seqT[kp][:, :, 32:160] if d == 0
                                    else seqT[kp][:, :, 0:128])
                            for nb in range(2):
                                nc.tensor.matmul(
                                    ps[0:128, nb * 512:(nb + 1) * 512],
                                    stat,
                                    WT[kp][:, :, q * 1024 + nb * 512:
                                           q * 1024 + (nb + 1) * 512],
                                    start=(d == 0 and kp == 0),
                                    stop=(d == 1 and kp == 3),
                                    perf_mode=DR, skip_group_check=True)
                    if use_bias:
                        # blt is pre-scaled by scl on the host
                        nc.vector.scalar_tensor_tensor(
                            zxq[q][:], ps[:], 1.0,
                            blt[:, q * 1024:(q + 1) * 1024],
                            mybir.AluOpType.mult, mybir.AluOpType.add)
                    elif evac_eng == 0:
                        nc.scalar.copy(zxq[q][:], ps[:])
                    else:
                        nc.vector.tensor_copy(zxq[q][:], ps[:])
                    return ps

                c_prev = None

                def chain_tail(t, si, sf, sg, so):
                    # c/h chain on merged [36,1024] tiles (both dirs)
                    nonlocal c_prev
                    c_new = lp.tile([36, 1024], BF16, tag="lc", bufs=2,
                                    name=f"c{t}")
                    if t > 0:
                        t1 = lp.tile([36, 1024], BF16, tag="ltmp", bufs=10,
                                     name=f"t1_{t}")
                        nc.vector.tensor_mul(t1[:], si[:], sg[:])
                        t2 = lp.tile([36, 1024], BF16, tag="ltmp", bufs=10,
                                     name=f"t2_{t}")
                        nc.vector.tensor_mul(t2[:], sf[:], c_prev[:])
                        nc.vector.tensor_add(c_new[:], t1[:], t2[:])
                    else:
                        nc.vector.tensor_mul(c_new[:], si[:], sg[:])
                    c_prev = c_new
                    tch = lp.tile([36, 1024], BF16, tag="ltmp", bufs=10,
                                  name=f"tc{t}")
                    nc.scalar.activation(tch[:], c_new[:], AF.Tanh)
                    ht = lp.tile([36, 1024], BF16, tag="lh", bufs=2,
                                 name=f"h{t}")
                    nc.vector.tensor_mul(ht[:], so[:], tch[:])
                    return ht

                def txp(t, ht):
                    # h_t (both dirs) -> Ht col band t+1: 16 PE transposes
                    # into one psum tile, then one copy per dir
                    tpp = lps.tile([128, 2, 8, 4], BF16, tag="ptr", bufs=2,
                                   name=f"tp{t}")
                    for d in range(2):
                        idb = ident8[32 * d:32 * d + 4, 32 * d:32 * d + 4]
                        for j in range(8):
                            nc.tensor.matmul(
                                tpp[:, d, j, :],
                                ht[32 * d:32 * d + 4, j * 128:(j + 1) * 128],
                                idb, is_transpose=True,
                                skip_group_check=True)
                    for d in range(2):
                        c0 = 32 * d + 4 * (t + 1)
                        dst = Ht[d][:, :, :, c0:c0 + 4]
                        src = tpp[:, d].rearrange("p (jp ko) b -> p jp ko b",
                                                  ko=2)
                        if d == 0:
                            nc.scalar.copy(dst, src)
                        else:
                            nc.vector.tensor_copy(dst, src)

                def act_q(t, q, ps, name):
                    g = lp.tile([36, 1024], BF16, tag="ltmp", bufs=10,
                                name=f"{name}{t}")
                    fn = AF.Tanh if q == 2 else AF.Sigmoid
                    nc.scalar.activation(g[:], ps[0:36, :], fn, scale=scinv)
                    return g

                # t=0: gates read the z_x psums directly (q=1/f unused);
                # q=1 runs on the PE while the t0 chain drains. With bias the
                # gates read zxq (psum + scaled bias) instead.
                def t0_src(q, ps):
                    return zxq[q] if use_bias else ps

                ps0 = zx_q(0, 0)
                si = act_q(0, 0, t0_src(0, ps0), "si")
                ps2 = zx_q(2, 1)
                sg = act_q(0, 2, t0_src(2, ps2), "sg")
                ps3 = zx_q(3, 0)
                so = act_q(0, 3, t0_src(3, ps3), "so")
                zx_q(1, 1)
                h = chain_tail(0, si, None, sg, so)
                txp(0, h)

                # ---- recurrence steps 1..3: per (q): fold z_x via the shift
                # matmul (starts the psum group), then h@U fp8 DR for both
                # dirs; gates read the psum bands directly. The q<2 folds of
                # the NEXT step are emitted before this step's transposes
                # (they only need zxq) to keep the PE fed through the gate
                # chain tail.
                pzh = [None] * 4

                def fold(t, q):
                    pz = lps.tile([128, 1024], F32, tag="pz", bufs=3,
                                  name=f"pzu{t}{q}")
                    for nb in range(2):
                        nc.tensor.matmul(
                            pz[0:128, nb * 512:(nb + 1) * 512],
                            SHt[:, t - 1, :],
                            zxq[q][:, nb * 512:(nb + 1) * 512],
                            start=True, stop=False,
                            skip_group_check=True)
                    return pz

                for t in range(1, 4):
                    gq = [None] * 4
                    names = ("si", "sf", "sg", "so")
                    for q in range(4):
                        pz = pzh[q] if pzh[q] is not None else fold(t, q)
                        pzh[q] = None
                        for d in (1, 0):
                            for kp in range(4):
                                stat = Ht[d][:, kp, :, 4 * t:4 * t + 128]
                                for nb in range(2):
                                    nc.tensor.matmul(
                                        pz[0:128, nb * 512:(nb + 1) * 512],
                                        stat,
                                        UL[d][kp][:, :, q * 1024 + nb * 512:
                                                  q * 1024 + (nb + 1) * 512],
                                        start=False,
                                        stop=(d == 0 and kp == 3 and nb == 1),
                                        perf_mode=DR, skip_group_check=True)
                        if q != 3:
                            gq[q] = act_q(t, q, pz, names[q])
                        else:
                            # emit tanh(c) before sig(o) on the Scalar queue
                            t1 = lp.tile([36, 1024], BF16, tag="ltmp",
                                         bufs=10, name=f"t1_{t}")
                            nc.vector.tensor_mul(t1[:], gq[0][:], gq[2][:])
                            t2 = lp.tile([36, 1024], BF16, tag="ltmp",
                                         bufs=10, name=f"t2_{t}")
                            nc.vector.tensor_mul(t2[:], gq[1][:], c_prev[:])
                            c_new = lp.tile([36, 1024], BF16, tag="lc",
                                            bufs=2, name=f"c{t}")
                            nc.vector.tensor_add(c_new[:], t1[:], t2[:])
                            c_prev = c_new
                            tch = lp.tile([36, 1024], BF16, tag="ltmp",
                                          bufs=10, name=f"tc{t}")
                            nc.scalar.activation(tch[:], c_new[:], AF.Tanh)
                            so = act_q(t, q, pz, names[q])
                            h = lp.tile([36, 1024], BF16, tag="lh", bufs=2,
                                        name=f"h{t}")
                            nc.vector.tensor_mul(h[:], so[:], tch[:])
                    if t < 3:
                        for q in range(2):
                            pzh[q] = fold(t + 1, q)
                    txp(t, h)

                if dbg == 'hs':
                    for d in range(2):
                        nc.sync.dma_start(dbg_ap[d], Ht[d][:])

                # ---- ffwd 1x1 conv + leaky relu -> d0 interior
                for mc in range(2):
                    pf = lps.tile([128, 64], F32, tag="ptr", bufs=2,
                                  name=f"pf{mc}")
                    for s in range(4):
                        for kc in range(4):
                            d, chalf = kc // 2, kc % 2
                            ud = s + 1 if d == 0 else 4 - s
                            c0 = 32 * d + 4 * ud
                            rhs = Ht[d][:, :, chalf, c0:c0 + 4]
                            nc.tensor.matmul(pf[:, s * 16:(s + 1) * 16],
                                             wffsb[:, kc, mc, :], rhs,
                                             start=(kc == 0), stop=(kc == 3))
                    t1 = lp.tile([128, 64], BF16, tag="lff", bufs=4,
                                 name=f"ff{mc}")
                    nc.scalar.activation(t1[:], pf[:], AF.Identity,
                                         bias=bias_sb[:, 16 + mc:17 + mc],
                                         scale=bias_sb[:, 28:29])
                    t2 = lp.tile([128, 64], BF16, tag="lff", bufs=4,
                                 name=f"fm{mc}")
                    nc.vector.tensor_scalar_mul(t2[:], t1[:], 0.3)
                    dst = d0[:, mc, :, 1:5, 1:5].rearrange("p b h s -> p s h b")
                    t1v = t1.rearrange("p (s h b) -> p s h b", s=4, h=4)
                    t2v = t2.rearrange("p (s h b) -> p s h b", s=4, h=4)
                    nc.vector.tensor_max(dst, t1v, t2v)

        if dbg == 'd0':
            for mc in range(2):
                nc.sync.dma_start(dbg_ap[mc], d0[:, mc])

        # ================= decoder =================
        with tc.tile_pool(name="decp", bufs=1) as dp, \
             tc.tile_pool(name="decps", bufs=1, space="PSUM") as dpp:
            w8d23 = [dp.tile([128, 16, 2, 2, 128], F8, name=f"w8d{2 + l}")
                     for l in range(2)]
            for l in range(2):
                nc.scalar.dma_start(w8d23[l][:], w8[[2, 3][l]])
            w8d = w8d + w8d23
            d1 = dp.tile([128, 2, B, 10, 10], F8, tag="dchain", bufs=4,
                         name="d1m")
            d2 = dp.tile([128, 2, B, 18, 18], F8, tag="dchain", bufs=4,
                         name="d2m")
            d3 = dp.tile([128, 2, B, 34, 34], F8, tag="dchain", bufs=4,
                         name="d3m")
            for mc in range(2):
                memset_border(d1[:, mc], 10)
                memset_border(d2[:, mc], 18)
                memset_border(d3[:, mc], 34)

            def dec_layer_dr(wt, act_in, get_dst, Hin, bias_idx, sc_idx):
                N = B * Hin * Hin
                for mc in range(2):
                    for ph in range(2):
                        for pw in range(2):
                            ps = dpp.tile([128, N], F32, tag="dps", bufs=8,
                                          name=f"psd{Hin}_{mc}{ph}{pw}")
                            taps = [(dm, kh, dn, kw, ko)
                                    for (dm, kh) in ROW_TAPS[ph]
                                    for (dn, kw) in ROW_TAPS[pw]
                                    for ko in range(2)]
                            for i, (dm, kh, dn, kw, ko) in enumerate(taps):
                                rhs = act_in[:, ko, :,
                                             1 + dm:1 + dm + Hin,
                                             1 + dn:1 + dn + Hin]
                                nc.tensor.matmul(
                                    ps[:], wt[:, kh * 4 + kw, mc, ko, :],
                                    rhs, start=(i == 0), stop=(i == 7))
                            dst = get_dst(mc, ph, pw, Hin)
                            nc.scalar.activation(
                                dst, ps[:], AF.Relu,
                                bias=bias_sb[:, bias_idx + mc:bias_idx + mc + 1],
                                scale=bias_sb[:, sc_idx:sc_idx + 1])

            dec_layer_dr(w8d[0], d0,
                         lambda mc, ph, pw, Hin: d1[:, mc, :,
                                                    _stepped(1 + ph, Hin, 2),
                                                    _stepped(1 + pw, Hin, 2)],
                         4, 8, 29)
            dec_layer_dr(w8d[1], d1,
                         lambda mc, ph, pw, Hin: d2[:, mc, :,
                                                    _stepped(1 + ph, Hin, 2),
                                                    _stepped(1 + pw, Hin, 2)],
                         8, 10, 30)

            # dec L2 (fp8 DR): per-b, weight-stationary over 4 b-psums
            Hin = 16
            for mc in range(2):
                for ph in range(2):
                    for pw in range(2):
                        pss = [dpp.tile([128, 256], F32, tag="dps", bufs=8,
                                        name=f"psd16_{mc}{ph}{pw}_{b}")
                               for b in range(B)]
                        taps = [(dm, kh, dn, kw)
                                for (dm, kh) in ROW_TAPS[ph]
                                for (dn, kw) in ROW_TAPS[pw]]
                        for i, (dm, kh, dn, kw) in enumerate(taps):
                            for b in range(B):
                                rhs = d2[:, :, b,
                                         1 + dm:1 + dm + Hin,
                                         1 + dn:1 + dn + Hin]
                                nc.tensor.matmul(
                                    pss[b][:], w8d[2][:, kh * 4 + kw, mc, :, :],
                                    rhs, start=(i == 0), stop=(i == 3),
                                    perf_mode=DR)
                        for b in range(B):
                            dst = d3[:, mc, b,
                                     _stepped(1 + ph, Hin, 2),
                                     _stepped(1 + pw, Hin, 2)]
                            # evacs alternate engines; the DVE form is
                            # max(psum + bias*sc, 0) with the 1/sc (power of
                            # 2) deferred into the final layer's evac scale
                            if b % 2 == 0:
                                nc.vector.tensor_scalar(
                                    dst, pss[b][:],
                                    bias_sb[:, 31 + mc:32 + mc],
                                    0.0, mybir.AluOpType.add,
                                    mybir.AluOpType.max)
                            else:
                                nc.scalar.activation(
                                    dst, pss[b][:], AF.Relu,
                                    bias=bias_sb[:, 31 + mc:32 + mc])

            if dbg in ('d1',):
                for mc in range(2):
                    nc.sync.dma_start(dbg_ap[mc], d1[:, mc])

            # final layer (fp8 DR) + residual, streamed per (b, mc, row
            # half) so the output DMA starts before the tile is fully done
            for b in range(B):
                for mc in range(2):
                    xr = dp.tile([128, 64, 64], BF16, tag="resid", bufs=4,
                                 name=f"xr{b}_{mc}")
                    nc.sync.dma_start(xr[:], xres[mc, :, b])
                    ob = dp.tile([128, 64, 64], BF16, tag="resid", bufs=4,
                                 name=f"ob{b}_{mc}")
                    for mh in range(2):
                        m0 = mh * 16
                        for ph in range(2):
                            for pw in range(2):
                                ps = dpp.tile([128, 512], F32, tag="dps",
                                              bufs=8,
                                              name=f"psf{b}{mc}{ph}{pw}{mh}")
                                taps = [(dm, kh, dn, kw)
                                        for (dm, kh) in ROW_TAPS[ph]
                                        for (dn, kw) in ROW_TAPS[pw]]
                                for i, (dm, kh, dn, kw) in enumerate(taps):
                                    rhs = d3[:, :, b,
                                             1 + dm + m0:1 + dm + m0 + 16,
                                             1 + dn:1 + dn + 32]
                                    nc.tensor.matmul(
                                        ps[:], w8d[3][:, kh * 4 + kw, mc, :, :],
                                        rhs, start=(i == 0), stop=(i == 3),
                                        perf_mode=DR)
                                t1 = dp.tile([128, 512], BF16, tag="fin",
                                             bufs=3, name=f"f{b}{mc}{ph}{pw}{mh}")
                                nc.scalar.activation(t1[:], ps[:], AF.Relu,
                                                     bias=bias_sb[:, 14 + mc:15 + mc],
                                                     scale=bias_sb[:, 23:24])
                                oslice = ob[:, _stepped(ph + 2 * m0, 16, 2),
                                            _stepped(pw, 32, 2)]
                                xslice = xr[:, _stepped(ph + 2 * m0, 16, 2),
                                            _stepped(pw, 32, 2)]
                                t1v = t1.rearrange("p (m n) -> p m n", m=16)
                                if gamma_nonneg:
                                    nc.vector.tensor_add(oslice, t1v, xslice)
                                else:
                                    nc.vector.tensor_sub(oslice, xslice, t1v)
                        nc.sync.dma_start(out[mc, :, b, 32 * mh:32 * mh + 32],
                                          ob[:, 32 * mh:32 * mh + 32])


# --------------------------------------------------------------------------
# host-side prep + entry point
# --------------------------------------------------------------------------

def _fold_bn(w, cb, g, bb, m, v):
    A = g / np.sqrt(v + BN_EPS)
    bias = (cb - m) * A + bb
    return w * A[None, None, None, :], bias


def prep_inputs(d):
    x = np.asarray(d['x'], np.float32)
    gamma = float(np.asarray(d['gamma']).reshape(-1)[0])
    g_abs, g_nonneg = abs(gamma), gamma >= 0

    def fold(pfx, l):
        g = np.asarray(d[f'{pfx}_bn_g'][l], np.float32)
        bb = np.asarray(d[f'{pfx}_bn_b'][l], np.float32)
        m = np.asarray(d[f'{pfx}_bn_m'][l], np.float32)
        v = np.asarray(d[f'{pfx}_bn_v'][l], np.float32)
        A = g / np.sqrt(v + BN_EPS)
        bias = (np.asarray(d[f'{pfx}_b'][l], np.float32) - m) * A + bb
        return np.asarray(d[f'{pfx}_w'][l], np.float32) * A[None, None, None, :], bias

    folded = {}
    for l in range(4):
        folded[('enc', l)] = fold('enc', l)
        w, bias = fold('dec', l)
        if l == 3:
            w, bias = w * g_abs, bias * g_abs
        folded[('dec', l)] = (w, bias)

    bconv = np.zeros((128, 34), np.float32)
    for l in range(4):
        bconv[:, l * 2] = folded[('enc', l)][1][:128]
        bconv[:, l * 2 + 1] = folded[('enc', l)][1][128:]
        bconv[:, 8 + l * 2] = folded[('dec', l)][1][:128]
        bconv[:, 8 + l * 2 + 1] = folded[('dec', l)][1][128:]
    bconv[:, 16] = np.asarray(d['ffwd_b'], np.float32)[:128]
    bconv[:, 17] = np.asarray(d['ffwd_b'], np.float32)[128:]

    # fp8 DoubleRow weights for all 8 conv layers
    # [encL1, encL2, decL2, decL3, encL3, encL4, decL0, decL1]
    w8 = np.zeros((8, 128, 16, 2, 2, 128), NPF8)
    W8_KEYS = (('enc', 0), ('enc', 1), ('dec', 2), ('dec', 3),
               ('enc', 2), ('enc', 3), ('dec', 0), ('dec', 1))
    W8_SC_COLS = (20, 21, 22, 23, 18, 19, 29, 30)
    scs = {}
    for i, key in enumerate(W8_KEYS):
        w, _ = folded[key]
        std = float(np.std(w)) + 1e-30
        sc = 2.0 ** round(np.log2(0.18 / std))
        scs[key] = sc
        ws = w * sc                                  # [4,4,Cin,Cout]
        # [ki, tap, mc, ko, m]: Cin = ko*128 + ki ; Cout = mc*128 + m
        # (entry 0 / enc L1 is mc-major: [ki, mc, tap, ko, m])
        tp = (3, 4, 0, 1, 2, 5) if i == 0 else (3, 0, 1, 4, 2, 5)
        w8[i] = (ws.reshape(4, 4, 2, 128, 2, 128)
                 .transpose(*tp)
                 .reshape(128, 16, 2, 2, 128).astype(NPF8))
        bconv[:, W8_SC_COLS[i]] = 1.0 / sc
    # dec L2 evac runs on DVE as max(psum + bias*sc, 0); its 1/sc moves
    # into the final layer's evac scale (col 23), and cols 31/32 hold the
    # pre-scaled biases
    bconv[:, 31] = bconv[:, 12] * scs[('dec', 2)]
    bconv[:, 32] = bconv[:, 13] * scs[('dec', 2)]
    bconv[:, 23] = 1.0 / (scs[('dec', 3)] * scs[('dec', 2)])

    def permW(w):
        # seq feature l = pix*64 + cc -> device row
        # l' = (pix//4)*256 + (pix%2)*128 + ((pix//2)%2)*64 + cc
        w4 = np.asarray(w).reshape(16, 64, 4096)
        out = np.empty((4, 2, 2, 64, 4096), w4.dtype)
        for pix in range(16):
            out[pix // 4, pix % 2, (pix // 2) % 2] = w4[pix]
        return np.ascontiguousarray(out.reshape(1024, 4096))

    wlf32 = [permW(np.asarray(d['lstm_fwd_W'], np.float32)),
             permW(np.asarray(d['lstm_rvs_W'], np.float32)),
             np.asarray(d['lstm_fwd_U'], np.float32),
             np.asarray(d['lstm_rvs_U'], np.float32)]
    # one shared power-of-2 scale for W and U so z_x and h@U psums add
    # scale-free (fp8 relative precision is scale-invariant here)
    stds = [float(np.std(m)) + 1e-30 for m in wlf32]
    scl = 2.0 ** round(np.log2(0.18 / float(np.exp(np.mean(np.log(stds))))))
    wl = np.zeros((4, 4, 128, 2, 4096), NPF8)
    for i, m in enumerate(wlf32):
        # row r = kp*256 + ko*128 + ki
        wl[i] = (m * scl).reshape(4, 2, 128, 4096).transpose(0, 2, 1, 3) \
                         .astype(NPF8)
    bconv[:, 26] = 1.0 / scl
    blv = np.stack([np.asarray(d['lstm_fwd_b'], np.float32),
                    np.asarray(d['lstm_rvs_b'], np.float32)])
    use_bias = bool(np.any(blv != 0))
    # pre-scaled by scl: raw-scale psums get bias added before the 1/scl
    # at the gate activations
    bl = np.broadcast_to(blv[:, None, :] * scl, (2, 16, 4096)).astype(NPBF).copy()

    wffv = np.asarray(d['ffwd_w'], np.float32)[0, 0]     # [512, 256]
    stdf = float(np.std(wffv)) + 1e-30
    scf = 2.0 ** round(np.log2(0.18 / stdf))
    bconv[:, 28] = 1.0 / scf
    wff = np.ascontiguousarray(
        (wffv * scf).reshape(4, 128, 2, 128).transpose(1, 0, 2, 3).astype(NPF8))

    xcm = np.zeros((N_CORES, 128, 2, B, 66, 66), NPF8)
    xrs = np.zeros((N_CORES, 2, 128, B, 64, 64), NPBF)
    xt = x.reshape(N_CORES, B, 64, 64, 2, 128).transpose(0, 4, 5, 1, 2, 3)
    xcm[:, :, :, :, 1:65, 1:65] = xt.transpose(0, 2, 1, 3, 4, 5).astype(NPF8)
    xrs[:] = xt.astype(NPBF)

    # shift matrices for the z_x fold (layout [128, 3, 128]: partition-major)
    shm = np.zeros((128, 3, 128), NPBF)
    for t in range(1, 4):
        for r in range(4):
            shm[4 * t + r, t - 1, r] = 1
            shm[32 + 4 * t + r, t - 1, 32 + r] = 1

    in_maps = []
    for c in range(N_CORES):
        in_maps.append(dict(xin=xcm[c], xres=xrs[c], w8=w8,
                            bconv=bconv, wl=wl, bl=bl, wff=wff, shm=shm))
    return in_maps, g_nonneg, use_bias


def get_nc(g_nonneg=True, use_bias=False, dbg=None):
    key = (g_nonneg, use_bias, dbg)
    if key not in _CACHE:
        _CACHE[key] = _build(gamma_nonneg=g_nonneg, use_bias=use_bias, dbg=dbg)
    return _CACHE[key]


def kernel(**inputs):
    in_maps, g_nonneg, use_bias = prep_inputs(inputs)
    nc = get_nc(g_nonneg, use_bias)
    res = run_bass_kernel_spmd(nc, in_maps, core_ids=list(range(N_CORES)))
    outs = []
    for c in range(N_CORES):
        o = np.asarray(res.results[c]["out"], np.float32)
        outs.append(o.transpose(2, 3, 4, 0, 1).reshape(B, 64, 64, 256))
    return np.concatenate(outs, axis=0).astype(np.float32)



# revision 56
# speedup vs baseline: 1.0075x; 1.0075x over previous
"""Trainium2 Bass kernel for nn_AFRM_48636209660262.

Conv-BiLSTM autoencoder: 4x strided conv encoder -> channel-split BiLSTM ->
1x1 conv ffwd -> 4x conv_transpose decoder -> gamma*h + x.

Strategy: pure data parallelism over 8 NeuronCores (4 samples each, no
collectives). Activations are channel-major [C_chunk(128), B, H, W] with
zero-padded borders; convs are per-tap matmuls accumulated in PSUM
(weight-stationary loop order so each LDWEIGHTS amortizes over 2-4 psum
tiles). All 8 conv layers run in fp8e4m3 (the four big ones - enc L1/L2,
dec L2/L3 - with perf_mode=DoubleRow, K=256 per matmul; the small middle
layers as plain fp8 with per-ko weight slices since their merged-b rhs APs
exceed the TENSOR3D pattern limit); per-layer power-of-2 weight scales are
undone via the evacuation's scale AP, BN folded into weights on the host,
bias + ReLU applied by ScalarE on evacuation. conv_transpose = 4 parity
classes x 4 taps. The enc4 -> LSTM feature shuffle happens on-chip: one
shift-by-64 permutation matmul per kc plus 8 partition-aligned DVE copies
build the DoubleRow-ready seqT stationary directly (no DRAM bounce); W rows
are host-permuted to match. z_x is precomputed for all timesteps into
per-(dir,q) tiles (evacuation alternates ScalarE/VectorE so t=0 gates start
early); the recurrence folds each round's z_x slice into the h@U psum group
with a scU-scaled 4x4 identity matmul, so gates read PSUM directly with the
shared 1/scU activation scale. h^T chunks come from small PE transposes
into per-chunk H tiles so the next round's matmuls start immediately.
Residual x and the output travel as bf16; PSUM accumulation is f32
throughout. Host-side prep only reshapes/quantizes weights and the input.
"""
import numpy as np
import ml_dtypes

import concourse.mybir as mybir
import concourse.tile as tile
from concourse import bacc
from concourse.bass_utils import run_bass_kernel_spmd
from concourse.masks import make_identity

AF = mybir.ActivationFunctionType
DR = mybir.MatmulPerfMode.DoubleRow
BF16 = mybir.dt.bfloat16
F32 = mybir.dt.float32
F8 = mybir.dt.float8e4
NPF8 = ml_dtypes.float8_e4m3fn
NPBF = ml_dtypes.bfloat16

N_CORES = 8
B = 4           # batch per core
C = 256
BN_EPS = 1e-3

_CACHE: dict = {}


def _stepped(start, count, step):
    return slice(start, start + step * (count - 1) + 1, step)


def _build(gamma_nonneg=True, use_bias=False, dbg=None):
    nc = bacc.Bacc("TRN2", target_bir_lowering=False, debug=False,
                   num_devices=N_CORES)

    xin = nc.dram_tensor("xin", [128, B, 2, 2, 34, 66], F8, kind="ExternalInput").ap()
    xres = nc.dram_tensor("xres", [2, 128, B, 64, 64], BF16, kind="ExternalInput").ap()
    # fp8 DoubleRow weights for all 8 conv layers
    # [encL1, encL2, decL2, decL3, encL3, encL4, decL0, decL1]
    w8 = nc.dram_tensor("w8", [8, 128, 16, 2, 2, 128], F8, kind="ExternalInput").ap()
    bconv = nc.dram_tensor("bconv", [128, 34], F32, kind="ExternalInput").ap()
    # LSTM mats: [Wf, Wr, Uf, Ur] x [kc, row, 4096]
    wl = nc.dram_tensor("wl", [4, 4, 128, 2, 4096], F8, kind="ExternalInput").ap()
    bl = nc.dram_tensor("bl", [2, 16, 4096], BF16, kind="ExternalInput").ap()
    wff = nc.dram_tensor("wff", [128, 4, 2, 128], F8, kind="ExternalInput").ap()
    shm = nc.dram_tensor("shm", [128, 3, 128], BF16, kind="ExternalInput").ap()
    out = nc.dram_tensor("out", [2, 128, B, 64, 64], BF16, kind="ExternalOutput").ap()

    dbg_ap = None
    dbg_shapes = {
        'l1': [2, 128, B, 34, 34], 'l2': [2, 128, B, 18, 18],
        'l3': [2, 128, B, 10, 10], 'enc4': [2, 128, B * 16],
        'seqT': [128, 4, 2, 160], 'hs': [2, 128, 4, 2, 32],
        'd0': [2, 128, B, 6, 6], 'd1': [2, 128, B, 10, 10],
        'd2': [2, 128, B, 18, 18], 'd3': [2, 128, B, 34, 34],
    }
    if dbg is not None:
        dbg_ap = nc.dram_tensor("dbg", dbg_shapes[dbg], BF16,
                                kind="ExternalOutput").ap()

    with tile.TileContext(nc) as tc:
        _trace(nc, tc, xin, xres, w8, bconv, wl, bl, wff, shm, out,
               gamma_nonneg, use_bias, dbg, dbg_ap)
    nc.compile()
    return nc


def _trace(nc, tc, xin, xres, w8, bconv, wl, bl, wff, shm, out,
           gamma_nonneg, use_bias, dbg, dbg_ap):
    from contextlib import ExitStack

    def memset_border(t, Hp):
        nc.vector.memset(t[:, :, 0, :], 0.0)
        nc.vector.memset(t[:, :, Hp - 1, :], 0.0)
        nc.vector.memset(t[:, :, :, 0], 0.0)
        nc.vector.memset(t[:, :, :, Hp - 1], 0.0)

    # decoder parity taps: out[2m+p] <- pairs (di, k)
    ROW_TAPS = {0: [(-1, 0), (0, 2)], 1: [(0, 1), (1, 3)]}

    with ExitStack() as top:
        persist = top.enter_context(tc.tile_pool(name="persist", bufs=1))

        bias_sb = persist.tile([128, 34], F32)
        nc.sync.dma_start(bias_sb[:], bconv[:])
        warm = persist.tile([1, 2], BF16, name="warm")
        nc.scalar.activation(warm[:, 0:1], bias_sb[0:1, 0:1], AF.Sigmoid)
        nc.scalar.activation(warm[:, 1:2], bias_sb[0:1, 0:1], AF.Tanh)
        ident8 = persist.tile([128, 128], BF16)
        make_identity(nc, ident8[:])
        # shift-by-64 permutation: SH[p, (p+64)%128] = 1
        shid = persist.tile([128, 128], BF16, name="shid")
        nc.vector.tensor_copy(shid[:, 64:128], ident8[:, 0:64])
        nc.vector.tensor_copy(shid[:, 0:64], ident8[:, 64:128])
        # h^T history per dir: [c, kp, ko, u*4+b] where col band u holds
        # h_{u-1} (band 0 = initial zeros); recurrence step t reads band t,
        # ffwd reads band t+1
        # Ht[1] has a 32-col zero prefix per (kp,ko): its z_u stationary is
        # M=48 (cols 0:32 zero) so the matmul lands d1 at psum rows 32:48
        # while keeping tile position 0 (DR + col-tile offset is invalid ISA)
        Ht = [persist.tile([128, 4, 2, 160], F8, name="hT0"),
              persist.tile([128, 4, 2, 160], F8, name="hT1")]
        for d in range(2):
            nc.vector.memset(Ht[d][:], 0.0)
        # shift matrices for the z_x fold: SH[t][4t+r, r] = 1 and
        # SH[t][32+4t+r, 32+r] = 1, so one bf16 matmul adds the step-t z_x
        # rows of both dirs into the gate psum bands at rows 0:4 / 32:36
        # (host-built: engine copies can't write partition base 4t)
        SHt = persist.tile([128, 3, 128], BF16, name="shm")
        nc.sync.dma_start(SHt[:], shm[:])
        enc4 = [persist.tile([128, 16, B], F8, name=f"enc4_{kc}")
                for kc in range(2)]  # [c, (hh,ww), b]
        d0 = persist.tile([128, 2, B, 6, 6], F8, name="d0m")
        wffsb = persist.tile([128, 4, 2, 128], F8)
        for mc in range(2):
            memset_border(d0[:, mc], 6)

        # decoder weights, all fp8 DR: [decL0, decL1] here; L2/L3 live in the
        # decoder pool (frees 16KB/partition during the LSTM phase)
        w8d = [persist.tile([128, 16, 2, 2, 128], F8, name=f"w8d{l}")
               for l in range(2)]

        # lwa pool spans encoder+lstm: W matrices prefetch during L2-L4
        # via Scalar-queue-gated triggers (full DMA bandwidth stays on the
        # critical x/w8 tiles during L1)
        with tc.tile_pool(name="lwa", bufs=1) as lwp:
            WLf = [lwp.tile([128, 2, 4096], F8, tag="lwa", bufs=4,
                            name=f"wf_{kp}") for kp in range(4)]
            WLr = [lwp.tile([128, 2, 4096], F8, tag="lwb", bufs=4,
                            name=f"wr_{kp}") for kp in range(4)]

            # ================= encoder =================
            with tc.tile_pool(name="encp", bufs=1) as ep, \
                 tc.tile_pool(name="encps", bufs=1, space="PSUM") as pp:
                # fp8 DoubleRow weights [L1, L2, L3, L4] <- w8[0,1,4,5];
                # L1's entry is host-packed mc-major so the first matmul
                # group only waits on the mc=0 half of the transfer
                w8t = [ep.tile([128, 16, 2, 2, 128], F8, tag="cw8", bufs=4,
                               name=f"w8e{l}") for l in range(1, 4)]
                w8t0 = ep.tile([128, 2, 16, 2, 128], F8, tag="cw8", bufs=4,
                               name="w8e0")
                w80v = w8[0].rearrange("p a c d m -> p (a c d m)").rearrange(
                    "p (mc t ko m) -> p mc t ko m", mc=2, t=16, ko=2)
                w8t = [None] + w8t
                # x tiles split per (b, row-half), merged ko dim; first-group
                # tiles (b0/b1 row-half 0) lead the DMA queues
                xt = [[ep.tile([128, 2, 34, 66], F8, tag="xcm", bufs=8,
                               name=f"x_{b}_{hf}") for hf in range(2)]
                      for b in range(B)]
                nc.sync.dma_start(xt[0][0][:], xin[:, 0, 0])
                nc.sync.dma_start(xt[1][0][:], xin[:, 1, 0])
                nc.sync.dma_start(w8t0[:, 0], w80v[:, 0])
                nc.sync.dma_start(w8t0[:, 1], w80v[:, 1])
                for b in range(B):
                    for hf in range(2):
                        if hf == 0 and b < 2:
                            continue
                        nc.sync.dma_start(xt[b][hf][:], xin[:, b, hf])
                nc.sync.dma_start(w8t[1][:], w8[1])

                l1 = ep.tile([128, 2, B, 34, 34], F8, tag="echain", bufs=4,
                             name="l1m")
                # l2/l3 parity-split: [c, ko, hp, wp, h', w', b] with b
                # innermost so the stride-2 conv reads collapse to 3 AP dims
                # and L3/L4 run DoubleRow with merged-b moving operands
                l2p = ep.tile([128, 2, 2, 2, 10, 10, B], F8, tag="echain",
                              bufs=4, name="l2p")
                l3p = ep.tile([128, 2, 2, 2, 6, 6, B], F8, tag="echain",
                              bufs=4, name="l3p")
                nc.vector.memset(l2p[:], 0.0)
                nc.vector.memset(l3p[:], 0.0)
                for ko in range(2):
                    memset_border(l1[:, ko], 34)

                # L1 (fp8 DR): weight-stationary, each weight streams 4
                # b-psums (2 for the first group, so the first matmuls only
                # wait on xt[0..1] + w8t[0])
                for hf, oh0 in ((0, 0), (1, 16)):
                    for mc in range(2):
                        bsets = ([(0, 1), (2, 3)] if hf == 0 and mc == 0
                                 else [(0, 1, 2, 3)])
                        for bset in bsets:
                            pss = {b: pp.tile([128, 512], F32, tag="cps",
                                              bufs=6,
                                              name=f"psl1_{hf}_{mc}_{b}")
                                   for b in bset}
                            for t in range(16):
                                kh, kw = t // 4, t % 4
                                for b in bset:
                                    rhs = xt[b][hf][:, :,
                                                   _stepped(kh, 16, 2),
                                                   _stepped(kw, 32, 2)]
                                    nc.tensor.matmul(
                                        pss[b][:], w8t0[:, mc, t, :, :], rhs,
                                        start=(t == 0), stop=(t == 15),
                                        perf_mode=DR)
                            for b in bset:
                                nc.scalar.activation(
                                    l1[:, mc, b, 1 + oh0:17 + oh0, 1:33],
                                    pss[b][:],
                                    AF.Relu, bias=bias_sb[:, mc:mc + 1],
                                    scale=bias_sb[:, 20:21])


                # deferred prefetches: issued after L1 in trace order so
                # they sit behind the critical transfers in the DMA queues
                nc.sync.dma_start(w8t[2][:], w8[4])
                nc.sync.dma_start(w8t[3][:], w8[5])
                for kp in range(4):
                    nc.sync.dma_start(WLf[kp][:], wl[0, kp])
                nc.sync.dma_start(wffsb[:], wff[:])
                for l in range(2):
                    nc.sync.dma_start(w8d[l][:], w8[[6, 7][l]])
                for kp in range(4):
                    nc.sync.dma_start(WLr[kp][:], wl[1, kp])

                # L2 (fp8 DR): per-b groups, weight-stationary over 4 b's
                for mc in range(2):
                    pss = [pp.tile([128, 256], F32, tag="cps", bufs=6,
                                   name=f"psl2_{mc}_{b}") for b in range(B)]
                    for t in range(16):
                        kh, kw = t // 4, t % 4
                        for b in range(B):
                            rhs = l1[:, :, b,
                                     _stepped(kh, 16, 2),
                                     _stepped(kw, 16, 2)]
                            nc.tensor.matmul(
                                pss[b][:], w8t[1][:, t, mc, :, :], rhs,
                                start=(t == 0), stop=(t == 15), perf_mode=DR)
                    for b in range(B):
                        psv = pss[b].rearrange(
                            "p (h a w c) -> p a c h w", h=8, a=2, w=8)
                        for eh in range(2):
                            nc.scalar.activation(
                                l2p[:, mc, eh, :, 1:9, 1:9, b], psv[:, eh],
                                AF.Relu, bias=bias_sb[:, 2 + mc:3 + mc],
                                scale=bias_sb[:, 21:22])

                # L3 (fp8 DR on parity layout): rhs [p, 2ko, h', (w'b)]
                l2f = l2p.rearrange("p k i j h w b -> p k i j h (w b)")
                for mc in range(2):
                    ps = pp.tile([128, 256], F32, tag="cps", bufs=6,
                                 name=f"psl3_{mc}")
                    for t in range(16):
                        kh, kw = t // 4, t % 4
                        rhs = l2f[:, :, 1 - kh % 2, 1 - kw % 2,
                                  (kh + 1) // 2:(kh + 1) // 2 + 8,
                                  4 * ((kw + 1) // 2):
                                  4 * ((kw + 1) // 2) + 32]
                        nc.tensor.matmul(
                            ps[:], w8t[2][:, t, mc, :, :], rhs,
                            start=(t == 0), stop=(t == 15), perf_mode=DR)
                    psv = ps.rearrange(
                        "p (h a w c b) -> p a c h w b", h=4, a=2, w=4, c=2)
                    for eh in range(2):
                        for ew in range(2):
                            nc.scalar.activation(
                                l3p[:, mc, eh, ew, 1:5, 1:5, :],
                                psv[:, eh, ew],
                                AF.Relu, bias=bias_sb[:, 4 + mc:5 + mc],
                                scale=bias_sb[:, 18:19])

                # L4 (fp8 DR on parity layout) -> enc4 [c, (hh ww), b]
                l3f = l3p.rearrange("p k i j h w b -> p k i j h (w b)")
                for mc in range(2):
                    ps = pp.tile([128, 64], F32, tag="cps", bufs=6,
                                 name=f"psl4_{mc}")
                    for t in range(16):
                        kh, kw = t // 4, t % 4
                        rhs = l3f[:, :, 1 - kh % 2, 1 - kw % 2,
                                  (kh + 1) // 2:(kh + 1) // 2 + 4,
                                  4 * ((kw + 1) // 2):
                                  4 * ((kw + 1) // 2) + 16]
                        nc.tensor.matmul(
                            ps[:], w8t[3][:, t, mc, :, :], rhs,
                            start=(t == 0), stop=(t == 15), perf_mode=DR)
                    nc.scalar.activation(
                        enc4[mc].rearrange("p hw b -> p (hw b)"),
                        ps[:], AF.Relu,
                        bias=bias_sb[:, 6 + mc:7 + mc],
                        scale=bias_sb[:, 19:20])

            if dbg == 'enc4':
                for kc in range(2):
                    nc.sync.dma_start(
                        dbg_ap[kc],
                        enc4[kc].rearrange("p hw b -> p (hw b)"))

            # ================= LSTM =================
            with tc.tile_pool(name="lstmp", bufs=1) as lp, \
                 tc.tile_pool(name="lstmps", bufs=1, space="PSUM") as lps:
                # on-chip enc4 -> seqT shuffle.  seqT row (band*64+cc) of
                # chunk (kp, ko) holds feature (hw=4kp+2band+ko, cc); cols
                # are (s,b) = (2kc+shi)*4+b.  W rows host-permuted to match.
                # Crossed half (shi != band) reads a 64-partition-swapped
                # copy made by one permutation matmul per kc.
                seqTm = lp.tile([128, 4, 2, 160], F8, name="seqTm")
                nc.vector.memset(seqTm[:], 0.0)
                e4sw = [lps.tile([128, 64], F32, tag="ptr", bufs=2,
                                 name=f"e4sw{kc}") for kc in range(2)]
                for kc in range(2):
                    nc.tensor.matmul(
                        e4sw[kc][:], shid[:],
                        enc4[kc].rearrange("p hw b -> p (hw b)"),
                        start=True, stop=True)
                for kc in range(2):
                    e4swv = e4sw[kc].rearrange("p (hw b) -> p hw b", b=B)
                    for band in range(2):
                        for am in range(2):
                            shi = band if am == 0 else 1 - band
                            s = 2 * kc + shi
                            src = (enc4[kc] if am == 0 else e4swv)
                            srcv = src[band * 64:(band + 1) * 64].rearrange(
                                "p (kp two ko) b -> p kp two ko b",
                                kp=4, two=2)[:, :, band, :, :]
                            dst = seqTm[band * 64:(band + 1) * 64, :, :,
                                        32 + s * 4:32 + s * 4 + 4]
                            # alternate engines so the 8 copies pipeline
                            # (z_x can't start until seqT is complete)
                            if (band + am) % 2 == 0:
                                nc.vector.tensor_copy(dst, srcv)
                            else:
                                nc.scalar.copy(dst, srcv)
                seqT = [seqTm[:, kp] for kp in range(4)]
                if dbg == 'seqT':
                    nc.sync.dma_start(dbg_ap[:], seqTm[:])

                # U matrices: dedicated buffers streamed during z_x / t0
                # gates. Triggered from the Scalar queue so the transfers
                # can't start before the encoder's last evacuation (running
                # them during the encoder slows its conv matmuls via SBUF
                # write contention).
                ULf = [lp.tile([128, 2, 4096], F8, tag="ula", bufs=4,
                               name=f"uf_{kp}") for kp in range(4)]
                ULr = [lp.tile([128, 2, 4096], F8, tag="ulb", bufs=4,
                               name=f"ur_{kp}") for kp in range(4)]
                UL = [ULf, ULr]
                nc.gpsimd.tensor_copy(warm[:, 1:2], enc4[0][0:1, 0, 0:1])
                for q in range(4):
                    for d in range(2):
                        for kp in range(4):
                            nc.gpsimd.dma_start(
                                UL[d][kp][:, :, q * 1024:(q + 1) * 1024],
                                wl[2 + d, kp][:, :, q * 1024:(q + 1) * 1024])

                # ---- z_x for all steps. Both dirs share one psum tile
                # (d0 rows 0:16, d1 rows 32:48 via matmul tile position), so
                # every evac/gate op covers both dirs in one instruction.
                # zxq holds RAW (sc-scaled) values; the 1/sc happens at the
                # gate activations.
                blt = None
                if use_bias:
                    blt = lp.tile([48, 4096], BF16, tag="zxbias", bufs=1,
                                  name="blt")
                    for d in range(2):
                        nc.sync.dma_start(blt[32 * d:32 * d + 16, :], bl[d])
                zxq = [lp.tile([128, 1024], BF16, tag="zxj", bufs=6,
                               name=f"zx{q}") for q in range(4)]
                for q in range(4):
                    nc.vector.memset(zxq[q][:], 0.0)
                scinv = bias_sb[0:36, 26:27]

                def zx_q(q, evac_eng):
                    ps = lps.tile([128, 1024], F32, tag="pz", bufs=3,
                                  name=f"pzx{q}")
                    for d in (0, 1):
                        WT = (WLf, WLr)[d]
                        for kp in range(4):
                            stat = (seqT[kp][:, :, 32:160] if d == 0
                                    else seqT[kp][:, :, 0:128])
                            for nb in range(2):
                                nc.tensor.matmul(
                                    ps[0:128, nb * 512:(nb + 1) * 512],
                                    stat,
                                    WT[kp][:, :, q * 1024 + nb * 512:
                                           q * 1024 + (nb + 1) * 512],
                                    start=(d == 0 and kp == 0),
                                    stop=(d == 1 and kp == 3),
                                    perf_mode=DR, skip_group_check=True)
                    if use_bias:
                        # blt is pre-scaled by scl on the host
                        nc.vector.scalar_tensor_tensor(
                            zxq[q][:], ps[:], 1.0,
                            blt[:, q * 1024:(q + 1) * 1024],
                            mybir.AluOpType.mult, mybir.AluOpType.add)
                    elif evac_eng == 0:
                        nc.scalar.copy(zxq[q][:], ps[:])
                    else:
                        nc.vector.tensor_copy(zxq[q][:], ps[:])
                    return ps

                c_prev = None

                def chain_tail(t, si, sf, sg, so):
                    # c/h chain on merged [36,1024] tiles (both dirs)
                    nonlocal c_prev
                    c_new = lp.tile([36, 1024], BF16, tag="lc", bufs=2,
                                    name=f"c{t}")
                    if t > 0:
                        t1 = lp.tile([36, 1024], BF16, tag="ltmp", bufs=10,
                                     name=f"t1_{t}")
                        nc.vector.tensor_mul(t1[:], si[:], sg[:])
                        t2 = lp.tile([36, 1024], BF16, tag="ltmp", bufs=10,
                                     name=f"t2_{t}")
                        nc.vector.tensor_mul(t2[:], sf[:], c_prev[:])
                        nc.vector.tensor_add(c_new[:], t1[:], t2[:])
                    else:
                        nc.vector.tensor_mul(c_new[:], si[:], sg[:])
                    c_prev = c_new
                    tch = lp.tile([36, 1024], BF16, tag="ltmp", bufs=10,
                                  name=f"tc{t}")
                    nc.scalar.activation(tch[:], c_new[:], AF.Tanh)
                    ht = lp.tile([36, 1024], BF16, tag="lh", bufs=2,
                                 name=f"h{t}")
                    nc.vector.tensor_mul(ht[:], so[:], tch[:])
                    return ht

                def txp(t, ht):
                    # h_t (both dirs) -> Ht col band t+1: 16 PE transposes
                    # into one psum tile, then one copy per dir
                    tpp = lps.tile([128, 2, 8, 4], BF16, tag="ptr", bufs=2,
                                   name=f"tp{t}")
                    for d in range(2):
                        idb = ident8[32 * d:32 * d + 4, 32 * d:32 * d + 4]
                        for j in range(8):
                            nc.tensor.matmul(
                                tpp[:, d, j, :],
                                ht[32 * d:32 * d + 4, j * 128:(j + 1) * 128],
                                idb, is_transpose=True,
                                skip_group_check=True)
                    for d in range(2):
                        c0 = 32 * d + 4 * (t + 1)
                        dst = Ht[d][:, :, :, c0:c0 + 4]
                        src = tpp[:, d].rearrange("p (jp ko) b -> p jp ko b",
                                                  ko=2)
                        if d == 0:
                            nc.scalar.copy(dst, src)
                        else:
                            nc.vector.tensor_copy(dst, src)

                def act_q(t, q, ps, name):
                    g = lp.tile([36, 1024], BF16, tag="ltmp", bufs=10,
                                name=f"{name}{t}")
                    fn = AF.Tanh if q == 2 else AF.Sigmoid
                    nc.scalar.activation(g[:], ps[0:36, :], fn, scale=scinv)
                    return g

                # t=0: gates read the z_x psums directly (q=1/f unused);
                # q=1 runs on the PE while the t0 chain drains. With bias the
                # gates read zxq (psum + scaled bias) instead.
                def t0_src(q, ps):
                    return zxq[q] if use_bias else ps

                ps0 = zx_q(0, 0)
                si = act_q(0, 0, t0_src(0, ps0), "si")
                ps2 = zx_q(2, 1)
                sg = act_q(0, 2, t0_src(2, ps2), "sg")
                ps3 = zx_q(3, 0)
                so = act_q(0, 3, t0_src(3, ps3), "so")
                zx_q(1, 1)
                h = chain_tail(0, si, None, sg, so)
                txp(0, h)

                # ---- recurrence steps 1..3: per (q): fold z_x via the shift
                # matmul (starts the psum group), then h@U fp8 DR for both
                # dirs; gates read the psum bands directly. The q<2 folds of
                # the NEXT step are emitted before this step's transposes
                # (they only need zxq) to keep the PE fed through the gate
                # chain tail.
                pzh = [None] * 4

                def fold(t, q):
                    pz = lps.tile([128, 1024], F32, tag="pz", bufs=3,
                                  name=f"pzu{t}{q}")
                    for nb in range(2):
                        nc.tensor.matmul(
                            pz[0:128, nb * 512:(nb + 1) * 512],
                            SHt[:, t - 1, :],
                            zxq[q][:, nb * 512:(nb + 1) * 512],
                            start=True, stop=False,
                            skip_group_check=True)
                    return pz

                for t in range(1, 4):
                    gq = [None] * 4
                    names = ("si", "sf", "sg", "so")
                    for q in range(4):
                        pz = pzh[q] if pzh[q] is not None else fold(t, q)
                        pzh[q] = None
                        for d in (1, 0):
                            for kp in range(4):
                                stat = Ht[d][:, kp, :, 4 * t:4 * t + 128]
                                for nb in range(2):
                                    nc.tensor.matmul(
                                        pz[0:128, nb * 512:(nb + 1) * 512],
                                        stat,
                                        UL[d][kp][:, :, q * 1024 + nb * 512:
                                                  q * 1024 + (nb + 1) * 512],
                                        start=False,
                                        stop=(d == 0 and kp == 3 and nb == 1),
                                        perf_mode=DR, skip_group_check=True)
                        if q != 3:
                            gq[q] = act_q(t, q, pz, names[q])
                        else:
                            # emit tanh(c) before sig(o) on the Scalar queue
                            t1 = lp.tile([36, 1024], BF16, tag="ltmp",
                                         bufs=10, name=f"t1_{t}")
                            nc.vector.tensor_mul(t1[:], gq[0][:], gq[2][:])
                            t2 = lp.tile([36, 1024], BF16, tag="ltmp",
                                         bufs=10, name=f"t2_{t}")
                            nc.vector.tensor_mul(t2[:], gq[1][:], c_prev[:])
                            c_new = lp.tile([36, 1024], BF16, tag="lc",
                                            bufs=2, name=f"c{t}")
                            nc.vector.tensor_add(c_new[:], t1[:], t2[:])
                            c_prev = c_new
                            tch = lp.tile([36, 1024], BF16, tag="ltmp",
                                          bufs=10, name=f"tc{t}")
                            nc.scalar.activation(tch[:], c_new[:], AF.Tanh)
                            so = act_q(t, q, pz, names[q])
                            h = lp.tile([36, 1024], BF16, tag="lh", bufs=2,
                                        name=f"h{t}")
                            nc.vector.tensor_mul(h[:], so[:], tch[:])
                    if t < 3:
                        for q in range(2):
                            pzh[q] = fold(t + 1, q)
                    txp(t, h)

                if dbg == 'hs':
                    for d in range(2):
                        nc.sync.dma_start(dbg_ap[d], Ht[d][:])

                # ---- ffwd 1x1 conv + leaky relu -> d0 interior
                for mc in range(2):
                    pf = lps.tile([128, 64], F32, tag="ptr", bufs=2,
                                  name=f"pf{mc}")
                    for s in range(4):
                        for kc in range(4):
                            d, chalf = kc // 2, kc % 2
                            ud = s + 1 if d == 0 else 4 - s
                            c0 = 32 * d + 4 * ud
                            rhs = Ht[d][:, :, chalf, c0:c0 + 4]
                            nc.tensor.matmul(pf[:, s * 16:(s + 1) * 16],
                                             wffsb[:, kc, mc, :], rhs,
                                             start=(kc == 0), stop=(kc == 3))
                    t1 = lp.tile([128, 64], BF16, tag="lff", bufs=4,
                                 name=f"ff{mc}")
                    nc.scalar.activation(t1[:], pf[:], AF.Identity,
                                         bias=bias_sb[:, 16 + mc:17 + mc],
                                         scale=bias_sb[:, 28:29])
                    t2 = lp.tile([128, 64], BF16, tag="lff", bufs=4,
                                 name=f"fm{mc}")
                    nc.vector.tensor_scalar_mul(t2[:], t1[:], 0.3)
                    dst = d0[:, mc, :, 1:5, 1:5].rearrange("p b h s -> p s h b")
                    t1v = t1.rearrange("p (s h b) -> p s h b", s=4, h=4)
                    t2v = t2.rearrange("p (s h b) -> p s h b", s=4, h=4)
                    nc.vector.tensor_max(dst, t1v, t2v)

        if dbg == 'd0':
            for mc in range(2):
                nc.sync.dma_start(dbg_ap[mc], d0[:, mc])

        # ================= decoder =================
        with tc.tile_pool(name="decp", bufs=1) as dp, \
             tc.tile_pool(name="decps", bufs=1, space="PSUM") as dpp:
            w8d23 = [dp.tile([128, 16, 2, 2, 128], F8, name=f"w8d{2 + l}")
                     for l in range(2)]
            for l in range(2):
                nc.scalar.dma_start(w8d23[l][:], w8[[2, 3][l]])
            w8d = w8d + w8d23
            d1 = dp.tile([128, 2, B, 10, 10], F8, tag="dchain", bufs=4,
                         name="d1m")
            d2 = dp.tile([128, 2, B, 18, 18], F8, tag="dchain", bufs=4,
                         name="d2m")
            d3 = dp.tile([128, 2, B, 34, 34], F8, tag="dchain", bufs=4,
                         name="d3m")
            for mc in range(2):
                memset_border(d1[:, mc], 10)
                memset_border(d2[:, mc], 18)
                memset_border(d3[:, mc], 34)

            def dec_layer_dr(wt, act_in, get_dst, Hin, bias_idx, sc_idx):
                N = B * Hin * Hin
                for mc in range(2):
                    for ph in range(2):
                        for pw in range(2):
                            ps = dpp.tile([128, N], F32, tag="dps", bufs=8,
                                          name=f"psd{Hin}_{mc}{ph}{pw}")
                            taps = [(dm, kh, dn, kw, ko)
                                    for (dm, kh) in ROW_TAPS[ph]
                                    for (dn, kw) in ROW_TAPS[pw]
                                    for ko in range(2)]
                            for i, (dm, kh, dn, kw, ko) in enumerate(taps):
                                rhs = act_in[:, ko, :,
                                             1 + dm:1 + dm + Hin,
                                             1 + dn:1 + dn + Hin]
                                nc.tensor.matmul(
                                    ps[:], wt[:, kh * 4 + kw, mc, ko, :],
                                    rhs, start=(i == 0), stop=(i == 7))
                            dst = get_dst(mc, ph, pw, Hin)
                            nc.scalar.activation(
                                dst, ps[:], AF.Relu,
                                bias=bias_sb[:, bias_idx + mc:bias_idx + mc + 1],
                                scale=bias_sb[:, sc_idx:sc_idx + 1])

            dec_layer_dr(w8d[0], d0,
                         lambda mc, ph, pw, Hin: d1[:, mc, :,
                                                    _stepped(1 + ph, Hin, 2),
                                                    _stepped(1 + pw, Hin, 2)],
                         4, 8, 29)
            dec_layer_dr(w8d[1], d1,
                         lambda mc, ph, pw, Hin: d2[:, mc, :,
                                                    _stepped(1 + ph, Hin, 2),
                                                    _stepped(1 + pw, Hin, 2)],
                         8, 10, 30)

            # dec L2 (fp8 DR): per-b, weight-stationary over 4 b-psums
            Hin = 16
            for mc in range(2):
                for ph in range(2):
                    for pw in range(2):
                        pss = [dpp.tile([128, 256], F32, tag="dps", bufs=8,
                                        name=f"psd16_{mc}{ph}{pw}_{b}")
                               for b in range(B)]
                        taps = [(dm, kh, dn, kw)
                                for (dm, kh) in ROW_TAPS[ph]
                                for (dn, kw) in ROW_TAPS[pw]]
                        for i, (dm, kh, dn, kw) in enumerate(taps):
                            for b in range(B):
                                rhs = d2[:, :, b,
                                         1 + dm:1 + dm + Hin,
                                         1 + dn:1 + dn + Hin]
                                nc.tensor.matmul(
                                    pss[b][:], w8d[2][:, kh * 4 + kw, mc, :, :],
                                    rhs, start=(i == 0), stop=(i == 3),
                                    perf_mode=DR)
                        for b in range(B):
                            dst = d3[:, mc, b,
                                     _stepped(1 + ph, Hin, 2),
                                     _stepped(1 + pw, Hin, 2)]
                            # evacs alternate engines; the DVE form is
                            # max(psum + bias*sc, 0) with the 1/sc (power of
                            # 2) deferred into the final layer's evac scale
                            if b % 2 == 0:
                                nc.vector.tensor_scalar(
                                    dst, pss[b][:],
                                    bias_sb[:, 31 + mc:32 + mc],
                                    0.0, mybir.AluOpType.add,
                                    mybir.AluOpType.max)
                            else:
                                nc.scalar.activation(
                                    dst, pss[b][:], AF.Relu,
                                    bias=bias_sb[:, 31 + mc:32 + mc])

            if dbg in ('d1',):
                for mc in range(2):
                    nc.sync.dma_start(dbg_ap[mc], d1[:, mc])

            # final layer (fp8 DR) + residual, streamed per (b, mc, row
            # half) so the output DMA starts before the tile is fully done
            for b in range(B):
                for mc in range(2):
                    xr = dp.tile([128, 64, 64], BF16, tag="resid", bufs=4,
                                 name=f"xr{b}_{mc}")
                    nc.sync.dma_start(xr[:], xres[mc, :, b])
                    ob = dp.tile([128, 64, 64], BF16, tag="resid", bufs=4,
                                 name=f"ob{b}_{mc}")
                    for mh in range(2):
                        m0 = mh * 16
                        for ph in range(2):
                            for pw in range(2):
                                ps = dpp.tile([128, 512], F32, tag="dps",
                                              bufs=8,
                                              name=f"psf{b}{mc}{ph}{pw}{mh}")
                                taps = [(dm, kh, dn, kw)
                                        for (dm, kh) in ROW_TAPS[ph]
                                        for (dn, kw) in ROW_TAPS[pw]]
                                for i, (dm, kh, dn, kw) in enumerate(taps):
                                    rhs = d3[:, :, b,
                                             1 + dm + m0:1 + dm + m0 + 16,
                                             1 + dn:1 + dn + 32]
                                    nc.tensor.matmul(
                                        ps[:], w8d[3][:, kh * 4 + kw, mc, :, :],
                                        rhs, start=(i == 0), stop=(i == 3),
                                        perf_mode=DR)
                                t1 = dp.tile([128, 512], BF16, tag="fin",
                                             bufs=3, name=f"f{b}{mc}{ph}{pw}{mh}")
                                nc.scalar.activation(t1[:], ps[:], AF.Relu,
                                                     bias=bias_sb[:, 14 + mc:15 + mc],
                                                     scale=bias_sb[:, 23:24])
                                oslice = ob[:, _stepped(ph + 2 * m0, 16, 2),
                                            _stepped(pw, 32, 2)]
                                xslice = xr[:, _stepped(ph + 2 * m0, 16, 2),
                                            _stepped(pw, 32, 2)]
                                t1v = t1.rearrange("p (m n) -> p m n", m=16)
                                if gamma_nonneg:
                                    nc.vector.tensor_add(oslice, t1v, xslice)
                                else:
                                    nc.vector.tensor_sub(oslice, xslice, t1v)
                        nc.sync.dma_start(out[mc, :, b, 32 * mh:32 * mh + 32],
                                          ob[:, 32 * mh:32 * mh + 32])


# --------------------------------------------------------------------------
# host-side prep + entry point
# --------------------------------------------------------------------------

def _fold_bn(w, cb, g, bb, m, v):
    A = g / np.sqrt(v + BN_EPS)
    bias = (cb - m) * A + bb
    return w * A[None, None, None, :], bias


def prep_inputs(d):
    x = np.asarray(d['x'], np.float32)
    gamma = float(np.asarray(d['gamma']).reshape(-1)[0])
    g_abs, g_nonneg = abs(gamma), gamma >= 0

    def fold(pfx, l):
        g = np.asarray(d[f'{pfx}_bn_g'][l], np.float32)
        bb = np.asarray(d[f'{pfx}_bn_b'][l], np.float32)
        m = np.asarray(d[f'{pfx}_bn_m'][l], np.float32)
        v = np.asarray(d[f'{pfx}_bn_v'][l], np.float32)
        A = g / np.sqrt(v + BN_EPS)
        bias = (np.asarray(d[f'{pfx}_b'][l], np.float32) - m) * A + bb
        return np.asarray(d[f'{pfx}_w'][l], np.float32) * A[None, None, None, :], bias

    folded = {}
    for l in range(4):
        folded[('enc', l)] = fold('enc', l)
        w, bias = fold('dec', l)
        if l == 3:
            w, bias = w * g_abs, bias * g_abs
        folded[('dec', l)] = (w, bias)

    bconv = np.zeros((128, 34), np.float32)
    for l in range(4):
        bconv[:, l * 2] = folded[('enc', l)][1][:128]
        bconv[:, l * 2 + 1] = folded[('enc', l)][1][128:]
        bconv[:, 8 + l * 2] = folded[('dec', l)][1][:128]
        bconv[:, 8 + l * 2 + 1] = folded[('dec', l)][1][128:]
    bconv[:, 16] = np.asarray(d['ffwd_b'], np.float32)[:128]
    bconv[:, 17] = np.asarray(d['ffwd_b'], np.float32)[128:]

    # fp8 DoubleRow weights for all 8 conv layers
    # [encL1, encL2, decL2, decL3, encL3, encL4, decL0, decL1]
    w8 = np.zeros((8, 128, 16, 2, 2, 128), NPF8)
    W8_KEYS = (('enc', 0), ('enc', 1), ('dec', 2), ('dec', 3),
               ('enc', 2), ('enc', 3), ('dec', 0), ('dec', 1))
    W8_SC_COLS = (20, 21, 22, 23, 18, 19, 29, 30)
    scs = {}
    for i, key in enumerate(W8_KEYS):
        w, _ = folded[key]
        std = float(np.std(w)) + 1e-30
        sc = 2.0 ** round(np.log2(0.18 / std))
        scs[key] = sc
        ws = w * sc                                  # [4,4,Cin,Cout]
        # [ki, tap, mc, ko, m]: Cin = ko*128 + ki ; Cout = mc*128 + m
        # (entry 0 / enc L1 is mc-major: [ki, mc, tap, ko, m])
        tp = (3, 4, 0, 1, 2, 5) if i == 0 else (3, 0, 1, 4, 2, 5)
        w8[i] = (ws.reshape(4, 4, 2, 128, 2, 128)
                 .transpose(*tp)
                 .reshape(128, 16, 2, 2, 128).astype(NPF8))
        bconv[:, W8_SC_COLS[i]] = 1.0 / sc
    # dec L2 evac runs on DVE as max(psum + bias*sc, 0); its 1/sc moves
    # into the final layer's evac scale (col 23), and cols 31/32 hold the
    # pre-scaled biases
    bconv[:, 31] = bconv[:, 12] * scs[('dec', 2)]
    bconv[:, 32] = bconv[:, 13] * scs[('dec', 2)]
    bconv[:, 23] = 1.0 / (scs[('dec', 3)] * scs[('dec', 2)])

    def permW(w):
        # seq feature l = pix*64 + cc -> device row
        # l' = (pix//4)*256 + (pix%2)*128 + ((pix//2)%2)*64 + cc
        w4 = np.asarray(w).reshape(16, 64, 4096)
        out = np.empty((4, 2, 2, 64, 4096), w4.dtype)
        for pix in range(16):
            out[pix // 4, pix % 2, (pix // 2) % 2] = w4[pix]
        return np.ascontiguousarray(out.reshape(1024, 4096))

    wlf32 = [permW(np.asarray(d['lstm_fwd_W'], np.float32)),
             permW(np.asarray(d['lstm_rvs_W'], np.float32)),
             np.asarray(d['lstm_fwd_U'], np.float32),
             np.asarray(d['lstm_rvs_U'], np.float32)]
    # one shared power-of-2 scale for W and U so z_x and h@U psums add
    # scale-free (fp8 relative precision is scale-invariant here)
    stds = [float(np.std(m)) + 1e-30 for m in wlf32]
    scl = 2.0 ** round(np.log2(0.18 / float(np.exp(np.mean(np.log(stds))))))
    wl = np.zeros((4, 4, 128, 2, 4096), NPF8)
    for i, m in enumerate(wlf32):
        # row r = kp*256 + ko*128 + ki
        wl[i] = (m * scl).reshape(4, 2, 128, 4096).transpose(0, 2, 1, 3) \
                         .astype(NPF8)
    bconv[:, 26] = 1.0 / scl
    blv = np.stack([np.asarray(d['lstm_fwd_b'], np.float32),
                    np.asarray(d['lstm_rvs_b'], np.float32)])
    use_bias = bool(np.any(blv != 0))
    # pre-scaled by scl: raw-scale psums get bias added before the 1/scl
    # at the gate activations
    bl = np.broadcast_to(blv[:, None, :] * scl, (2, 16, 4096)).astype(NPBF).copy()

    wffv = np.asarray(d['ffwd_w'], np.float32)[0, 0]     # [512, 256]
    stdf = float(np.std(wffv)) + 1e-30
    scf = 2.0 ** round(np.log2(0.18 / stdf))
    bconv[:, 28] = 1.0 / scf
    wff = np.ascontiguousarray(
        (wffv * scf).reshape(4, 128, 2, 128).transpose(1, 0, 2, 3).astype(NPF8))

    xpad = np.zeros((N_CORES, 128, 2, B, 66, 66), NPF8)
    xrs = np.zeros((N_CORES, 2, 128, B, 64, 64), NPBF)
    xt = x.reshape(N_CORES, B, 64, 64, 2, 128).transpose(0, 4, 5, 1, 2, 3)
    xpad[:, :, :, :, 1:65, 1:65] = xt.transpose(0, 2, 1, 3, 4, 5).astype(NPF8)
    xrs[:] = xt.astype(NPBF)
    # hf-major layout: each (b, hf) transfer is one contiguous 4.5KB run
    # per partition (vs 2.2KB strided), halving DMA descriptor count
    xcm = np.zeros((N_CORES, 128, B, 2, 2, 34, 66), NPF8)
    for hf in range(2):
        xcm[:, :, :, hf] = xpad[:, :, :, :, 32 * hf:32 * hf + 34, :] \
            .transpose(0, 1, 3, 2, 4, 5)

    # shift matrices for the z_x fold (layout [128, 3, 128]: partition-major)
    shm = np.zeros((128, 3, 128), NPBF)
    for t in range(1, 4):
        for r in range(4):
            shm[4 * t + r, t - 1, r] = 1
            shm[32 + 4 * t + r, t - 1, 32 + r] = 1

    in_maps = []
    for c in range(N_CORES):
        in_maps.append(dict(xin=xcm[c], xres=xrs[c], w8=w8,
                            bconv=bconv, wl=wl, bl=bl, wff=wff, shm=shm))
    return in_maps, g_nonneg, use_bias


def get_nc(g_nonneg=True, use_bias=False, dbg=None):
    key = (g_nonneg, use_bias, dbg)
    if key not in _CACHE:
        _CACHE[key] = _build(gamma_nonneg=g_nonneg, use_bias=use_bias, dbg=dbg)
    return _CACHE[key]


def kernel(**inputs):
    in_maps, g_nonneg, use_bias = prep_inputs(inputs)
    nc = get_nc(g_nonneg, use_bias)
    res = run_bass_kernel_spmd(nc, in_maps, core_ids=list(range(N_CORES)))
    outs = []
    for c in range(N_CORES):
        o = np.asarray(res.results[c]["out"], np.float32)
        outs.append(o.transpose(2, 3, 4, 0, 1).reshape(B, 64, 64, 256))
    return np.concatenate(outs, axis=0).astype(np.float32)

